# revision 30
# baseline (speedup 1.0000x reference)
"""Trainium2 Bass kernel for nn_DiscreteDistributionOutput (vq_codebook).

Math restructuring: the (b,k) distance matrix over K=64 candidates never needs
the (b,192,h,w) conv output materialized.  With F = feat (64, HW), e-rows
pl/t appended, the extended Gram A A^T (A = [F; pl; t], 70 x 70 per sample)
contains everything:
  dist[b,k] * N = sum_p w_kp^T G w_kp + 2 sum_p w_kp^T (G_Fpl - G_Ft)[:,p]
                  + ||pl - t||^2
All heavy compute is per-sample Gram accumulation on the PE (fp32), done via
PE-transposed 128-px chunks.  Distances, argmin, one-hot candidate selection
and the final predicts matmul (selected 3x64 weights, col-tiled across the 4
samples each core owns) all stay on-device.

Sharding: data-parallel over batch, 4 samples per core across 8 cores, no
cross-core communication (per sharding hint).
"""
import os
import sys

for _p in ("/opt/trn_rl_repo", "/root/.axon_site/_ro/trn_rl_repo"):
    if os.path.isdir(_p):
        if _p not in sys.path:
            sys.path.insert(0, _p)
        break

import numpy as np
from contextlib import ExitStack

import concourse.bacc as bacc
import concourse.mybir as mybir
from concourse import tile
from concourse.bass_utils import run_bass_kernel_spmd

dt = mybir.dt
Alu = mybir.AluOpType

B, C, H, W_IMG = 32, 64, 128, 128
HW = H * W_IMG            # 16384
K, PC = 64, 3
NCORES = 8
SPC = B // NCORES         # 4 samples per core
AR = C + 2 * PC           # 70 rows of A = [F; pl; t]
NMEAN = PC * HW           # 49152
SC = 2048                 # px superchunk
NSC = HW // SC            # 8
CPS = SC // 128           # 16 chunks of 128 px per superchunk

_CACHE = {}


def _build_nc():
    nc = bacc.Bacc("TRN2", target_bir_lowering=False, debug=False,
                   num_devices=NCORES)
    gdt = dt.float16

    a_hi = nc.dram_tensor("a_hi", [SPC, 128, HW], dt.float16, kind="ExternalInput").ap()
    feat = nc.dram_tensor("feat", [SPC, C, HW], dt.float32, kind="ExternalInput").ap()
    plq = nc.dram_tensor("plq", [SPC, PC, HW], dt.float32, kind="ExternalInput").ap()
    wt = nc.dram_tensor("wt", [C, K * PC], dt.float32, kind="ExternalInput").ap()
    wkp = nc.dram_tensor("wkp", [PC, K, C], dt.float32, kind="ExternalInput").ap()
    sel4 = nc.dram_tensor("sel4", [C, 4 * SPC], dt.float32, kind="ExternalInput").ap()
    vp = nc.dram_tensor("vp", [2 * PC, PC], dt.float32, kind="ExternalInput").ap()
    eye3 = nc.dram_tensor("eye3", [PC, PC], dt.float32, kind="ExternalInput").ap()

    pred_o = nc.dram_tensor("pred", [SPC, PC, HW], dt.float32, kind="ExternalOutput").ap()
    dm_o = nc.dram_tensor("dm", [SPC, K], dt.float32, kind="ExternalOutput").ap()
    idx_o = nc.dram_tensor("idx", [SPC, 1], dt.int32, kind="ExternalOutput").ap()
    loss_o = nc.dram_tensor("loss", [SPC, 1], dt.float32, kind="ExternalOutput").ap()
    dist_o = nc.dram_tensor("dists", [SPC, 1], dt.float32, kind="ExternalOutput").ap()

    with tile.TileContext(nc) as tc, ExitStack() as ctx:
        pool = ctx.enter_context(tc.tile_pool(name="sbuf", bufs=1))
        apool = ctx.enter_context(tc.tile_pool(name="ap", bufs=8))
        atpool = ctx.enter_context(tc.tile_pool(name="at", bufs=8))
        a2pool = ctx.enter_context(tc.tile_pool(name="a2", bufs=4))
        spool = ctx.enter_context(tc.tile_pool(name="sp", bufs=2))
        ppool = ctx.enter_context(tc.tile_pool(name="ps", bufs=3, space="PSUM"))
        gpool = ctx.enter_context(tc.tile_pool(name="gp", bufs=1, space="PSUM"))
        cpool = ctx.enter_context(tc.tile_pool(name="cp", bufs=2, space="PSUM"))

        # ---- constants in SBUF
        wtsb = pool.tile([C, K * PC], dt.float32, tag="wtsb")
        nc.sync.dma_start(wtsb[:], wt[:])
        wkpsb = pool.tile([C, PC * C], dt.float32, tag="wkpsb")
        for p in range(PC):
            nc.sync.dma_start(wkpsb[:, p * C:(p + 1) * C], wkp[p])
        sel4sb = pool.tile([C, 4 * SPC], dt.float32, tag="sel4sb")
        nc.sync.dma_start(sel4sb[:], sel4[:])
        vp64 = pool.tile([AR, PC], dt.float32, tag="vp64")
        nc.sync.dma_start(vp64[C:AR, :], vp[:])
        vp0 = pool.tile([2 * PC, PC], dt.float32, tag="vp0")
        nc.sync.dma_start(vp0[:], vp[:])

        id70 = pool.tile([128, AR], gdt, tag="id70")
        idt = pool.tile([128, AR], dt.int32, tag="idt")
        nc.gpsimd.iota(idt[:], pattern=[[-1, AR]], base=0, channel_multiplier=1)
        nc.vector.tensor_scalar(id70[:], idt[:], 0, None, op0=Alu.is_equal)
        id4 = pool.tile([SPC, SPC], dt.float32, tag="id4")
        idt4 = pool.tile([SPC, SPC], dt.int32, tag="idt4")
        nc.gpsimd.iota(idt4[:], pattern=[[-1, SPC]], base=0, channel_multiplier=1)
        nc.vector.tensor_scalar(id4[:], idt4[:], 0, None, op0=Alu.is_equal)

        # per-sample predicts lhsT = [WselT; I3]; eye rows preloaded
        wsel_sb = []
        for s in range(SPC):
            wtile = spool.tile([C + PC, PC], dt.float32, tag=f"wsel{s}", bufs=1)
            nc.sync.dma_start(wtile[C:C + PC, :], eye3[:])
            wsel_sb.append(wtile)

        # ---- phase A: per-sample extended Gram accumulation
        # BW: weight-block width, padded to 128 so LDWEIGHTS gets FastWeightLoad
        BW = 128
        gps = []
        for s in range(SPC):
            gtile = gpool.tile([BW, AR], dt.float32, tag=f"G{s % 2}")
            gps.append(gtile)

        # per-sample distance prep, emitted right after sample s's Gram so it
        # hides under the next sample's phase-A work (psum scratch borrows the
        # phase-C bank, idle during phase A)
        qtce_box = {}

        def emit_b_sample(s):
            gsb = spool.tile([AR, AR], dt.float32, tag="gsb", name=f"gsb{s}")
            nc.vector.tensor_copy(gsb[:], gps[s][0:AR, :])
            if s == 0:
                qtce_box["t"] = gpool.tile([SPC, 195], dt.float32, tag="QT",
                                           name="qtce")
            qtce = qtce_box["t"]
            m1ps = cpool.tile([C, K * PC], dt.float32, tag="pp", name=f"m1ps{s}")
            nc.tensor.matmul(m1ps[:], gsb[0:C, 0:C], wtsb[:], start=True, stop=True)
            m1sb = spool.tile([C, K * PC], dt.float32, tag="m1sb", name=f"m1sb{s}")
            nc.vector.tensor_copy(m1sb[:], m1ps[:])
            ub = spool.tile([C, PC], dt.float32, tag="ub", name=f"ub{s}")
            nc.vector.tensor_tensor(ub[:], gsb[0:C, C:C + PC], gsb[0:C, C + PC:AR],
                                    op=Alu.subtract)
            eevps = cpool.tile([2 * PC, PC], dt.float32, tag="pp", name=f"eevps{s}")
            nc.tensor.matmul(eevps[:], gsb[C:AR, C:AR], vp64[C:AR, :],
                             start=True, stop=True)
            eevsb = spool.tile([2 * PC, PC], dt.float32, tag="eevsb", name=f"eevsb{s}")
            nc.vector.tensor_copy(eevsb[:], eevps[:])
            eevv = spool.tile([2 * PC, PC], dt.float32, tag="eevv", name=f"eevv{s}")
            nc.vector.tensor_tensor(eevv[:], eevsb[:], vp0[:], op=Alu.mult)
            tm3 = spool.tile([C, 195], dt.float32, tag="tm3", name=f"tm3{s}")
            ub_b = ub[:].unsqueeze(1).broadcast_to([C, K, PC])
            nc.vector.scalar_tensor_tensor(
                tm3[:, 0:K * PC], ub_b, 2.0, m1sb[:], op0=Alu.mult, op1=Alu.add)
            nc.vector.tensor_tensor(tm3[:, 0:K * PC], tm3[:, 0:K * PC], wtsb[:],
                                    op=Alu.mult)
            nc.vector.memset(tm3[:, K * PC:195], 0.0)
            nc.vector.tensor_copy(tm3[0:2 * PC, K * PC:195], eevv[:])
            nc.tensor.matmul(qtce[:], sel4sb[:, SPC * s:SPC * (s + 1)], tm3[:],
                             start=(s == 0), stop=(s == SPC - 1))

        with nc.named_scope("phaseA"):
            for s in range(SPC):
                for sc in range(NSC):
                    a_t = apool.tile([128, SC], dt.float16, tag="A")
                    # Alternate gpsimd/scalar queues (both idle during phase
                    # A, and off the Sync FIFO that phase-C loads can block).
                    # a_hi is host-padded to 128 rows: full-partition DMAs
                    # engage all 16 SDMA ports (~4x the 70-partition rate).
                    if sc % 2 == 0:
                        nc.gpsimd.dma_start(a_t[:], a_hi[s][:, sc * SC:(sc + 1) * SC])
                    else:
                        nc.scalar.dma_start(a_t[:], a_hi[s][:, sc * SC:(sc + 1) * SC])
                    for q in range(CPS // 8):
                        # 8 transposed chunks fill the 2KB psum bank exactly;
                        # halves the copy count vs 4-chunk batches
                        p_t = ppool.tile([128, 8 * BW], gdt, tag="pT")
                        for j in range(8):
                            ch = q * 8 + j
                            nc.tensor.transpose(
                                p_t[:, j * BW:j * BW + AR],
                                a_t[:, ch * 128:(ch + 1) * 128], id70[:])
                        at_t = atpool.tile([128, 8 * BW], gdt, tag="AT")
                        # alternate the PSUM->SBUF copies between DVE and ACT
                        # so neither engine sits on the critical chain alone
                        if q % 3 != 2:
                            nc.vector.tensor_copy(at_t[:], p_t[:])
                        else:
                            nc.scalar.copy(at_t[:], p_t[:])
                        first = (sc == 0 and q == 0)
                        last = (sc == NSC - 1 and q == CPS // 8 - 1)
                        for j in range(8):
                            nc.tensor.matmul(
                                gps[s][0:BW, :],
                                at_t[:, j * BW:(j + 1) * BW],
                                at_t[:, j * BW:j * BW + AR],
                                start=(first and j == 0),
                                stop=(last and j == 7))
                emit_b_sample(s)

        # ---- phase B: distances, argmin, Wsel
        with nc.named_scope("phaseB"):
            qtce = qtce_box["t"]
            qkce = spool.tile([SPC, 65], dt.float32, tag="qkce")
            nc.vector.tensor_reduce(qkce[:], qtce[:].rearrange("a (k p) -> a k p", p=PC),
                                    axis=mybir.AxisListType.X, op=Alu.add)
            dneg = spool.tile([SPC, K], dt.float32, tag="dneg")
            nc.vector.tensor_scalar(dneg[:], qkce[:, 0:K], qkce[:, K:K + 1],
                                    -1.0 / NMEAN, op0=Alu.add, op1=Alu.mult)
            dmsb = spool.tile([SPC, K], dt.float32, tag="dmsb")
            nc.vector.tensor_scalar(dmsb[:], dneg[:], -1.0, None, op0=Alu.mult)
            nc.gpsimd.dma_start(dm_o[:], dmsb[:])

            mx8 = spool.tile([SPC, 8], dt.float32, tag="mx8")
            ix8 = spool.tile([SPC, 8], dt.uint32, tag="ix8")
            nc.vector.max(mx8[:], dneg[:])
            nc.vector.max_index(ix8[:], mx8[:], dneg[:])
            mn = spool.tile([SPC, 1], dt.float32, tag="mn")
            nc.vector.tensor_scalar(mn[:], mx8[:, 0:1], -1.0, None, op0=Alu.mult)
            nc.gpsimd.dma_start(loss_o[:], mn[:])
            nc.gpsimd.dma_start(dist_o[:], mn[:])
            idx32 = spool.tile([SPC, 1], dt.int32, tag="idx32")
            nc.vector.tensor_copy(idx32[:], ix8[:, 0:1])
            nc.gpsimd.dma_start(idx_o[:], idx32[:])

            oh = spool.tile([SPC, K], dt.float32, tag="oh")
            nc.vector.tensor_scalar(oh[:], dneg[:], mx8[:, 0:1], None, op0=Alu.is_equal)
            ohtps = ppool.tile([K, SPC], dt.float32, tag="pT")
            nc.tensor.transpose(ohtps[:], oh[:], id4[:])
            ohts = spool.tile([K, SPC], dt.float32, tag="ohts")
            nc.vector.tensor_copy(ohts[:], ohtps[:])

            # WselT per sample: 12 tiny matmuls into one bank
            wps = ppool.tile([C, PC * SPC], dt.float32, tag="pT")
            for s in range(SPC):
                for p in range(PC):
                    nc.tensor.matmul(wps[:, s * PC + p:s * PC + p + 1],
                                     wkpsb[:, p * C:(p + 1) * C],
                                     ohts[:, s:s + 1], start=True, stop=True)
            for s in range(SPC):
                nc.vector.tensor_copy(wsel_sb[s][0:C, :], wps[:, s * PC:(s + 1) * PC])

        # ---- phase C: predicts, fp32, col-tiled across the 4 samples
        with nc.named_scope("phaseC"):
            for sc in range(NSC):
                a2 = []
                for s in range(SPC):
                    a2t = a2pool.tile([C + PC, SC], dt.float32, tag=f"A2_{s}")
                    nc.sync.dma_start(a2t[0:C, :], feat[s][:, sc * SC:(sc + 1) * SC])
                    nc.sync.dma_start(a2t[C:C + PC, :], plq[s][:, sc * SC:(sc + 1) * SC])
                    a2.append(a2t)
                psb = spool.tile([99, SC], dt.float32, tag="psb")
                for w in range(SC // 512):
                    pp = cpool.tile([99, 512], dt.float32, tag="pp")
                    for s in range(SPC):
                        nc.tensor.matmul(
                            pp[32 * s:32 * s + PC, :], wsel_sb[s][:],
                            a2[s][:, w * 512:(w + 1) * 512],
                            start=True, stop=True, tile_position=(0, 32 * s),
                            skip_group_check=True)
                    nc.vector.tensor_copy(psb[:, w * 512:(w + 1) * 512], pp[:])
                off = sc * SC
                for s in range(SPC):
                    nc.scalar.dma_start(pred_o[s][:, off:off + SC],
                                        psb[32 * s:32 * s + PC, :])

    nc.compile()
    return nc


def _get_nc():
    if "nc" not in _CACHE:
        _CACHE["nc"] = _build_nc()
    return _CACHE["nc"]


def kernel(feat_last, target, predict_last, weight):
    feat_last = np.ascontiguousarray(np.asarray(feat_last, dtype=np.float32))
    target = np.ascontiguousarray(np.asarray(target, dtype=np.float32))
    predict_last = np.ascontiguousarray(np.asarray(predict_last, dtype=np.float32))
    weight = np.ascontiguousarray(np.asarray(weight, dtype=np.float32))

    featr = feat_last.reshape(B, C, HW)
    plr = predict_last.reshape(B, PC, HW)
    a_hi = np.zeros((B, 128, HW), dtype=np.float16)
    a_hi[:, 0:C] = featr
    a_hi[:, C:C + PC] = plr
    a_hi[:, C + PC:AR] = target.reshape(B, PC, HW)

    wt_np = np.ascontiguousarray(weight.T)                       # (64, 192)
    wkp_np = np.ascontiguousarray(
        weight.reshape(K, PC, C).transpose(1, 0, 2))             # (3, 64, 64)
    sel4_np = np.zeros((C, 4 * SPC), dtype=np.float32)
    for s in range(SPC):
        sel4_np[:, SPC * s + s] = 1.0
    vp_np = np.concatenate([np.eye(PC), -np.eye(PC)], axis=0).astype(np.float32)
    eye3_np = np.eye(PC, dtype=np.float32)

    in_maps = []
    for c in range(NCORES):
        sl = slice(c * SPC, (c + 1) * SPC)
        in_maps.append({
            "a_hi": np.ascontiguousarray(a_hi[sl]),
            "feat": np.ascontiguousarray(featr[sl]),
            "plq": np.ascontiguousarray(plr[sl]),
            "wt": wt_np, "wkp": wkp_np, "sel4": sel4_np,
            "vp": vp_np, "eye3": eye3_np,
        })

    nc = _get_nc()
    trace = os.environ.get("KERNEL_TRACE", "0") == "1"
    res = None
    last_err = None
    for attempt in range(3):
        try:
            res = run_bass_kernel_spmd(nc, in_maps, list(range(NCORES)),
                                       trace=trace,
                                       trace_cores=[0] if trace else None)
            break
        except Exception as e:  # transient NRT_EXEC_UNIT_UNRECOVERABLE right after
            last_err = e        # another process released the device; retry
            import time as _time
            _time.sleep(3.0 * (attempt + 1))
    if res is None:
        raise last_err
    if trace and res.exec_time_ns is not None:
        print(f"HW exec time: {res.exec_time_ns} ns")
        _CACHE["last_result"] = res

    predicts = np.concatenate([r["pred"] for r in res.results], axis=0)
    dm = np.concatenate([r["dm"] for r in res.results], axis=0)
    idx = np.concatenate([r["idx"][:, 0] for r in res.results], axis=0).astype(np.int32)
    loss = np.concatenate([r["loss"][:, 0] for r in res.results], axis=0)
    dists = np.concatenate([r["dists"][:, 0] for r in res.results], axis=0)
    return (predicts.reshape(B, PC, H, W_IMG), dm, idx, loss, dists)


# revision 32
# speedup vs baseline: 1.0007x; 1.0007x over previous
"""Trainium2 Bass kernel for nn_DiscreteDistributionOutput (vq_codebook).

Math restructuring: the (b,k) distance matrix over K=64 candidates never needs
the (b,192,h,w) conv output materialized.  With F = feat (64, HW), e-rows
pl/t appended, the extended Gram A A^T (A = [F; pl; t], 70 x 70 per sample)
contains everything:
  dist[b,k] * N = sum_p w_kp^T G w_kp + 2 sum_p w_kp^T (G_Fpl - G_Ft)[:,p]
                  + ||pl - t||^2
All heavy compute is per-sample Gram accumulation on the PE (fp32), done via
PE-transposed 128-px chunks.  Distances, argmin, one-hot candidate selection
and the final predicts matmul (selected 3x64 weights, col-tiled across the 4
samples each core owns) all stay on-device.

Sharding: data-parallel over batch, 4 samples per core across 8 cores, no
cross-core communication (per sharding hint).
"""
import os
import sys

for _p in ("/opt/trn_rl_repo", "/root/.axon_site/_ro/trn_rl_repo"):
    if os.path.isdir(_p):
        if _p not in sys.path:
            sys.path.insert(0, _p)
        break

import numpy as np
from contextlib import ExitStack

import concourse.bacc as bacc
import concourse.mybir as mybir
from concourse import tile
from concourse.bass_utils import run_bass_kernel_spmd

dt = mybir.dt
Alu = mybir.AluOpType

B, C, H, W_IMG = 32, 64, 128, 128
HW = H * W_IMG            # 16384
K, PC = 64, 3
NCORES = 8
SPC = B // NCORES         # 4 samples per core
AR = C + 2 * PC           # 70 rows of A = [F; pl; t]
NMEAN = PC * HW           # 49152
SC = 2048                 # px superchunk
NSC = HW // SC            # 8
CPS = SC // 128           # 16 chunks of 128 px per superchunk

_CACHE = {}


def _build_nc():
    nc = bacc.Bacc("TRN2", target_bir_lowering=False, debug=False,
                   num_devices=NCORES)
    gdt = dt.float16

    a_hi = nc.dram_tensor("a_hi", [SPC, 128, HW], dt.float16, kind="ExternalInput").ap()
    feat = nc.dram_tensor("feat", [SPC, C, HW], dt.float32, kind="ExternalInput").ap()
    plq = nc.dram_tensor("plq", [SPC, PC, HW], dt.float32, kind="ExternalInput").ap()
    wt = nc.dram_tensor("wt", [C, K * PC], dt.float32, kind="ExternalInput").ap()
    wkp = nc.dram_tensor("wkp", [PC, K, C], dt.float32, kind="ExternalInput").ap()
    sel4 = nc.dram_tensor("sel4", [C, 4 * SPC], dt.float32, kind="ExternalInput").ap()
    vp = nc.dram_tensor("vp", [2 * PC, PC], dt.float32, kind="ExternalInput").ap()
    eye3 = nc.dram_tensor("eye3", [PC, PC], dt.float32, kind="ExternalInput").ap()

    pred_o = nc.dram_tensor("pred", [SPC, PC, HW], dt.float32, kind="ExternalOutput").ap()
    dm_o = nc.dram_tensor("dm", [SPC, K], dt.float32, kind="ExternalOutput").ap()
    idx_o = nc.dram_tensor("idx", [SPC, 1], dt.int32, kind="ExternalOutput").ap()
    loss_o = nc.dram_tensor("loss", [SPC, 1], dt.float32, kind="ExternalOutput").ap()
    dist_o = nc.dram_tensor("dists", [SPC, 1], dt.float32, kind="ExternalOutput").ap()

    with tile.TileContext(nc) as tc, ExitStack() as ctx:
        pool = ctx.enter_context(tc.tile_pool(name="sbuf", bufs=1))
        apool = ctx.enter_context(tc.tile_pool(name="ap", bufs=8))
        atpool = ctx.enter_context(tc.tile_pool(name="at", bufs=8))
        a2pool = ctx.enter_context(tc.tile_pool(name="a2", bufs=4))
        spool = ctx.enter_context(tc.tile_pool(name="sp", bufs=2))
        ppool = ctx.enter_context(tc.tile_pool(name="ps", bufs=3, space="PSUM"))
        gpool = ctx.enter_context(tc.tile_pool(name="gp", bufs=1, space="PSUM"))
        cpool = ctx.enter_context(tc.tile_pool(name="cp", bufs=2, space="PSUM"))

        # ---- constants in SBUF
        wtsb = pool.tile([C, K * PC], dt.float32, tag="wtsb")
        nc.sync.dma_start(wtsb[:], wt[:])
        wkpsb = pool.tile([C, PC * C], dt.float32, tag="wkpsb")
        for p in range(PC):
            nc.sync.dma_start(wkpsb[:, p * C:(p + 1) * C], wkp[p])
        sel4sb = pool.tile([C, 4 * SPC], dt.float32, tag="sel4sb")
        nc.sync.dma_start(sel4sb[:], sel4[:])
        vp64 = pool.tile([AR, PC], dt.float32, tag="vp64")
        nc.sync.dma_start(vp64[C:AR, :], vp[:])
        vp0 = pool.tile([2 * PC, PC], dt.float32, tag="vp0")
        nc.sync.dma_start(vp0[:], vp[:])

        id70 = pool.tile([128, AR], gdt, tag="id70")
        idt = pool.tile([128, AR], dt.int32, tag="idt")
        nc.gpsimd.iota(idt[:], pattern=[[-1, AR]], base=0, channel_multiplier=1)
        nc.vector.tensor_scalar(id70[:], idt[:], 0, None, op0=Alu.is_equal)
        id4 = pool.tile([SPC, SPC], dt.float32, tag="id4")
        idt4 = pool.tile([SPC, SPC], dt.int32, tag="idt4")
        nc.gpsimd.iota(idt4[:], pattern=[[-1, SPC]], base=0, channel_multiplier=1)
        nc.vector.tensor_scalar(id4[:], idt4[:], 0, None, op0=Alu.is_equal)

        # per-sample predicts lhsT = [WselT; I3]; eye rows preloaded
        wsel_sb = []
        for s in range(SPC):
            wtile = spool.tile([C + PC, PC], dt.float32, tag=f"wsel{s}", bufs=1)
            nc.sync.dma_start(wtile[C:C + PC, :], eye3[:])
            wsel_sb.append(wtile)

        # ---- phase A: per-sample extended Gram accumulation
        # BW: weight-block width, padded to 128 so LDWEIGHTS gets FastWeightLoad
        BW = 128
        gps = []
        for s in range(SPC):
            gtile = gpool.tile([BW, AR], dt.float32, tag=f"G{s % 2}")
            gps.append(gtile)

        # per-sample distance prep, emitted right after sample s's Gram so it
        # hides under the next sample's phase-A work (psum scratch borrows the
        # phase-C bank, idle during phase A)
        qtce_box = {}

        def emit_b_sample(s):
            gsb = spool.tile([AR, AR], dt.float32, tag="gsb", name=f"gsb{s}")
            nc.vector.tensor_copy(gsb[:], gps[s][0:AR, :])
            if s == 0:
                qtce_box["t"] = gpool.tile([SPC, 195], dt.float32, tag="QT",
                                           name="qtce")
            qtce = qtce_box["t"]
            m1ps = cpool.tile([C, K * PC], dt.float32, tag="pp", name=f"m1ps{s}")
            nc.tensor.matmul(m1ps[:], gsb[0:C, 0:C], wtsb[:], start=True, stop=True)
            m1sb = spool.tile([C, K * PC], dt.float32, tag="m1sb", name=f"m1sb{s}")
            nc.vector.tensor_copy(m1sb[:], m1ps[:])
            ub = spool.tile([C, PC], dt.float32, tag="ub", name=f"ub{s}")
            nc.vector.tensor_tensor(ub[:], gsb[0:C, C:C + PC], gsb[0:C, C + PC:AR],
                                    op=Alu.subtract)
            eevps = cpool.tile([2 * PC, PC], dt.float32, tag="pp", name=f"eevps{s}")
            nc.tensor.matmul(eevps[:], gsb[C:AR, C:AR], vp64[C:AR, :],
                             start=True, stop=True)
            eevsb = spool.tile([2 * PC, PC], dt.float32, tag="eevsb", name=f"eevsb{s}")
            nc.vector.tensor_copy(eevsb[:], eevps[:])
            eevv = spool.tile([2 * PC, PC], dt.float32, tag="eevv", name=f"eevv{s}")
            nc.vector.tensor_tensor(eevv[:], eevsb[:], vp0[:], op=Alu.mult)
            tm3 = spool.tile([C, 195], dt.float32, tag="tm3", name=f"tm3{s}")
            ub_b = ub[:].unsqueeze(1).broadcast_to([C, K, PC])
            nc.vector.scalar_tensor_tensor(
                tm3[:, 0:K * PC], ub_b, 2.0, m1sb[:], op0=Alu.mult, op1=Alu.add)
            nc.vector.tensor_tensor(tm3[:, 0:K * PC], tm3[:, 0:K * PC], wtsb[:],
                                    op=Alu.mult)
            nc.vector.memset(tm3[:, K * PC:195], 0.0)
            nc.vector.tensor_copy(tm3[0:2 * PC, K * PC:195], eevv[:])
            nc.tensor.matmul(qtce[:], sel4sb[:, SPC * s:SPC * (s + 1)], tm3[:],
                             start=(s == 0), stop=(s == SPC - 1))

        with nc.named_scope("phaseA"):
            for s in range(SPC):
                for sc in range(NSC):
                    a_t = apool.tile([128, SC], dt.float16, tag="A")
                    # Alternate gpsimd/scalar queues (both idle during phase
                    # A, and off the Sync FIFO that phase-C loads can block).
                    # a_hi is host-padded to 128 rows: full-partition DMAs
                    # engage all 16 SDMA ports (~4x the 70-partition rate).
                    if sc % 2 == 0:
                        nc.gpsimd.dma_start(a_t[:], a_hi[s][:, sc * SC:(sc + 1) * SC])
                    else:
                        nc.scalar.dma_start(a_t[:], a_hi[s][:, sc * SC:(sc + 1) * SC])
                    for q in range(CPS // 8):
                        # 8 transposed chunks fill the 2KB psum bank exactly;
                        # halves the copy count vs 4-chunk batches
                        p_t = ppool.tile([128, 8 * BW], gdt, tag="pT")
                        for j in range(8):
                            ch = q * 8 + j
                            nc.tensor.transpose(
                                p_t[:, j * BW:j * BW + AR],
                                a_t[:, ch * 128:(ch + 1) * 128], id70[:])
                        at_t = atpool.tile([128, 8 * BW], gdt, tag="AT")
                        # alternate the PSUM->SBUF copies between DVE and ACT
                        # so neither engine sits on the critical chain alone
                        if q % 3 != 2:
                            nc.vector.tensor_copy(at_t[:], p_t[:])
                        else:
                            nc.scalar.copy(at_t[:], p_t[:])
                        first = (sc == 0 and q == 0)
                        last = (sc == NSC - 1 and q == CPS // 8 - 1)
                        for j in range(8):
                            nc.tensor.matmul(
                                gps[s][0:BW, :],
                                at_t[:, j * BW:(j + 1) * BW],
                                at_t[:, j * BW:j * BW + AR],
                                start=(first and j == 0),
                                stop=(last and j == 7))
                emit_b_sample(s)

        # ---- phase B: distances, argmin, Wsel
        with nc.named_scope("phaseB"):
            qtce = qtce_box["t"]
            qkce = spool.tile([SPC, 65], dt.float32, tag="qkce")
            nc.vector.tensor_reduce(qkce[:], qtce[:].rearrange("a (k p) -> a k p", p=PC),
                                    axis=mybir.AxisListType.X, op=Alu.add)
            dneg = spool.tile([SPC, K], dt.float32, tag="dneg")
            nc.vector.tensor_scalar(dneg[:], qkce[:, 0:K], qkce[:, K:K + 1],
                                    -1.0 / NMEAN, op0=Alu.add, op1=Alu.mult)
            dmsb = spool.tile([SPC, K], dt.float32, tag="dmsb")
            nc.vector.tensor_scalar(dmsb[:], dneg[:], -1.0, None, op0=Alu.mult)
            nc.gpsimd.dma_start(dm_o[:], dmsb[:])

            mx8 = spool.tile([SPC, 8], dt.float32, tag="mx8")
            ix8 = spool.tile([SPC, 8], dt.uint32, tag="ix8")
            nc.vector.max(mx8[:], dneg[:])
            nc.vector.max_index(ix8[:], mx8[:], dneg[:])
            mn = spool.tile([SPC, 1], dt.float32, tag="mn")
            nc.vector.tensor_scalar(mn[:], mx8[:, 0:1], -1.0, None, op0=Alu.mult)
            nc.gpsimd.dma_start(loss_o[:], mn[:])
            nc.gpsimd.dma_start(dist_o[:], mn[:])
            idx32 = spool.tile([SPC, 1], dt.int32, tag="idx32")
            nc.vector.tensor_copy(idx32[:], ix8[:, 0:1])
            nc.gpsimd.dma_start(idx_o[:], idx32[:])

            oh = spool.tile([SPC, K], dt.float32, tag="oh")
            nc.vector.tensor_scalar(oh[:], dneg[:], mx8[:, 0:1], None, op0=Alu.is_equal)
            ohtps = ppool.tile([K, SPC], dt.float32, tag="pT")
            nc.tensor.transpose(ohtps[:], oh[:], id4[:])
            ohts = spool.tile([K, SPC], dt.float32, tag="ohts")
            nc.vector.tensor_copy(ohts[:], ohtps[:])

            # WselT per sample: 12 tiny matmuls into one bank
            wps = ppool.tile([C, PC * SPC], dt.float32, tag="pT")
            for s in range(SPC):
                for p in range(PC):
                    nc.tensor.matmul(wps[:, s * PC + p:s * PC + p + 1],
                                     wkpsb[:, p * C:(p + 1) * C],
                                     ohts[:, s:s + 1], start=True, stop=True)
            for s in range(SPC):
                nc.vector.tensor_copy(wsel_sb[s][0:C, :], wps[:, s * PC:(s + 1) * PC])

        # ---- phase C: predicts, fp32, col-tiled across the 4 samples
        with nc.named_scope("phaseC"):
            for sc in range(NSC):
                a2 = []
                for s in range(SPC):
                    a2t = a2pool.tile([C + PC, SC], dt.float32, tag=f"A2_{s}")
                    nc.sync.dma_start(a2t[0:C, :], feat[s][:, sc * SC:(sc + 1) * SC])
                    nc.sync.dma_start(a2t[C:C + PC, :], plq[s][:, sc * SC:(sc + 1) * SC])
                    a2.append(a2t)
                psb = spool.tile([99, SC], dt.float32, tag="psb")
                for w in range(SC // 512):
                    pp = cpool.tile([99, 512], dt.float32, tag="pp")
                    for s in range(SPC):
                        nc.tensor.matmul(
                            pp[32 * s:32 * s + PC, :], wsel_sb[s][:],
                            a2[s][:, w * 512:(w + 1) * 512],
                            start=True, stop=True, tile_position=(0, 32 * s),
                            skip_group_check=True)
                    nc.vector.tensor_copy(psb[:, w * 512:(w + 1) * 512], pp[:])
                off = sc * SC
                for s in range(SPC):
                    nc.scalar.dma_start(pred_o[s][:, off:off + SC],
                                        psb[32 * s:32 * s + PC, :])

    nc.compile()
    return nc


def _get_nc():
    if "nc" not in _CACHE:
        _CACHE["nc"] = _build_nc()
    return _CACHE["nc"]


def kernel(feat_last, target, predict_last, weight):
    feat_last = np.ascontiguousarray(np.asarray(feat_last, dtype=np.float32))
    target = np.ascontiguousarray(np.asarray(target, dtype=np.float32))
    predict_last = np.ascontiguousarray(np.asarray(predict_last, dtype=np.float32))
    weight = np.ascontiguousarray(np.asarray(weight, dtype=np.float32))

    featr = feat_last.reshape(B, C, HW)
    plr = predict_last.reshape(B, PC, HW)
    a_hi = np.zeros((B, 128, HW), dtype=np.float16)
    a_hi[:, 0:C] = featr
    a_hi[:, C:C + PC] = plr
    a_hi[:, C + PC:AR] = target.reshape(B, PC, HW)

    wt_np = np.ascontiguousarray(weight.T)                       # (64, 192)
    wkp_np = np.ascontiguousarray(
        weight.reshape(K, PC, C).transpose(1, 0, 2))             # (3, 64, 64)
    sel4_np = np.zeros((C, 4 * SPC), dtype=np.float32)
    for s in range(SPC):
        sel4_np[:, SPC * s + s] = 1.0
    vp_np = np.concatenate([np.eye(PC), -np.eye(PC)], axis=0).astype(np.float32)
    eye3_np = np.eye(PC, dtype=np.float32)

    in_maps = []
    for c in range(NCORES):
        sl = slice(c * SPC, (c + 1) * SPC)
        in_maps.append({
            "a_hi": np.ascontiguousarray(a_hi[sl]),
            "feat": np.ascontiguousarray(featr[sl]),
            "plq": np.ascontiguousarray(plr[sl]),
            "wt": wt_np, "wkp": wkp_np, "sel4": sel4_np,
            "vp": vp_np, "eye3": eye3_np,
        })

    nc = _get_nc()
    trace = os.environ.get("KERNEL_TRACE", "0") == "1"
    res = None
    last_err = None
    for attempt in range(3):
        try:
            res = run_bass_kernel_spmd(nc, in_maps, list(range(NCORES)),
                                       trace=trace,
                                       trace_cores=[0] if trace else None)
            break
        except Exception as e:  # transient NRT_EXEC_UNIT_UNRECOVERABLE right after
            last_err = e        # another process released the device; retry
            import time as _time
            _time.sleep(3.0 * (attempt + 1))
    if res is None:
        raise last_err
    if trace and res.exec_time_ns is not None:
        print(f"HW exec time: {res.exec_time_ns} ns")
        _CACHE["last_result"] = res

    predicts = np.concatenate([r["pred"] for r in res.results], axis=0)
    dm = np.concatenate([r["dm"] for r in res.results], axis=0)
    idx = np.concatenate([r["idx"][:, 0] for r in res.results], axis=0).astype(np.int32)
    loss = np.concatenate([r["loss"][:, 0] for r in res.results], axis=0)
    dists = np.concatenate([r["dists"][:, 0] for r in res.results], axis=0)
    return (predicts.reshape(B, PC, H, W_IMG), dm, idx, loss, dists)


# revision 33
# speedup vs baseline: 1.0051x; 1.0043x over previous
"""Trainium2 Bass kernel for nn_DiscreteDistributionOutput (vq_codebook).

Math restructuring: the (b,k) distance matrix over K=64 candidates never needs
the (b,192,h,w) conv output materialized.  With F = feat (64, HW), e-rows
pl/t appended, the extended Gram A A^T (A = [F; pl; t], 70 x 70 per sample)
contains everything:
  dist[b,k] * N = sum_p w_kp^T G w_kp + 2 sum_p w_kp^T (G_Fpl - G_Ft)[:,p]
                  + ||pl - t||^2
All heavy compute is per-sample Gram accumulation on the PE (fp32), done via
PE-transposed 128-px chunks.  Distances, argmin, one-hot candidate selection
and the final predicts matmul (selected 3x64 weights, col-tiled across the 4
samples each core owns) all stay on-device.

Sharding: data-parallel over batch, 4 samples per core across 8 cores, no
cross-core communication (per sharding hint).
"""
import os
import sys

for _p in ("/opt/trn_rl_repo", "/root/.axon_site/_ro/trn_rl_repo"):
    if os.path.isdir(_p):
        if _p not in sys.path:
            sys.path.insert(0, _p)
        break

import numpy as np
from contextlib import ExitStack

import concourse.bacc as bacc
import concourse.mybir as mybir
from concourse import tile
from concourse.bass_utils import run_bass_kernel_spmd

dt = mybir.dt
Alu = mybir.AluOpType

B, C, H, W_IMG = 32, 64, 128, 128
HW = H * W_IMG            # 16384
K, PC = 64, 3
NCORES = 8
SPC = B // NCORES         # 4 samples per core
AR = C + 2 * PC           # 70 rows of A = [F; pl; t]
NMEAN = PC * HW           # 49152
SC = 2048                 # px superchunk
NSC = HW // SC            # 8
CPS = SC // 128           # 16 chunks of 128 px per superchunk

_CACHE = {}


def _build_nc():
    nc = bacc.Bacc("TRN2", target_bir_lowering=False, debug=False,
                   num_devices=NCORES)
    gdt = dt.float16

    a_hi = nc.dram_tensor("a_hi", [SPC, 128, HW], dt.float16, kind="ExternalInput").ap()
    feat = nc.dram_tensor("feat", [SPC, C, HW], dt.float32, kind="ExternalInput").ap()
    plq = nc.dram_tensor("plq", [SPC, PC, HW], dt.float32, kind="ExternalInput").ap()
    wt = nc.dram_tensor("wt", [C, K * PC], dt.float32, kind="ExternalInput").ap()
    wkp = nc.dram_tensor("wkp", [PC, K, C], dt.float32, kind="ExternalInput").ap()
    sel4 = nc.dram_tensor("sel4", [C, 4 * SPC], dt.float32, kind="ExternalInput").ap()
    vp = nc.dram_tensor("vp", [2 * PC, PC], dt.float32, kind="ExternalInput").ap()
    eye3 = nc.dram_tensor("eye3", [PC, PC], dt.float32, kind="ExternalInput").ap()

    pred_o = nc.dram_tensor("pred", [SPC, PC, HW], dt.float32, kind="ExternalOutput").ap()
    dm_o = nc.dram_tensor("dm", [SPC, K], dt.float32, kind="ExternalOutput").ap()
    idx_o = nc.dram_tensor("idx", [SPC, 1], dt.int32, kind="ExternalOutput").ap()
    loss_o = nc.dram_tensor("loss", [SPC, 1], dt.float32, kind="ExternalOutput").ap()
    dist_o = nc.dram_tensor("dists", [SPC, 1], dt.float32, kind="ExternalOutput").ap()

    with tile.TileContext(nc) as tc, ExitStack() as ctx:
        pool = ctx.enter_context(tc.tile_pool(name="sbuf", bufs=1))
        apool = ctx.enter_context(tc.tile_pool(name="ap", bufs=8))
        atpool = ctx.enter_context(tc.tile_pool(name="at", bufs=8))
        a2pool = ctx.enter_context(tc.tile_pool(name="a2", bufs=4))
        spool = ctx.enter_context(tc.tile_pool(name="sp", bufs=2))
        ppool = ctx.enter_context(tc.tile_pool(name="ps", bufs=3, space="PSUM"))
        gpool = ctx.enter_context(tc.tile_pool(name="gp", bufs=1, space="PSUM"))
        cpool = ctx.enter_context(tc.tile_pool(name="cp", bufs=2, space="PSUM"))

        # ---- constants in SBUF
        wtsb = pool.tile([C, K * PC], dt.float32, tag="wtsb")
        nc.sync.dma_start(wtsb[:], wt[:])
        wkpsb = pool.tile([C, PC * C], dt.float32, tag="wkpsb")
        for p in range(PC):
            nc.sync.dma_start(wkpsb[:, p * C:(p + 1) * C], wkp[p])
        sel4sb = pool.tile([C, 4 * SPC], dt.float32, tag="sel4sb")
        nc.sync.dma_start(sel4sb[:], sel4[:])
        vp64 = pool.tile([AR, PC], dt.float32, tag="vp64")
        nc.sync.dma_start(vp64[C:AR, :], vp[:])
        vp0 = pool.tile([2 * PC, PC], dt.float32, tag="vp0")
        nc.sync.dma_start(vp0[:], vp[:])

        id70 = pool.tile([128, AR], gdt, tag="id70")
        idt = pool.tile([128, AR], dt.int32, tag="idt")
        nc.gpsimd.iota(idt[:], pattern=[[-1, AR]], base=0, channel_multiplier=1)
        nc.vector.tensor_scalar(id70[:], idt[:], 0, None, op0=Alu.is_equal)
        id4 = pool.tile([SPC, SPC], dt.float32, tag="id4")
        idt4 = pool.tile([SPC, SPC], dt.int32, tag="idt4")
        nc.gpsimd.iota(idt4[:], pattern=[[-1, SPC]], base=0, channel_multiplier=1)
        nc.vector.tensor_scalar(id4[:], idt4[:], 0, None, op0=Alu.is_equal)

        # per-sample predicts lhsT = [WselT; I3]; eye rows preloaded
        wsel_sb = []
        for s in range(SPC):
            wtile = spool.tile([C + PC, PC], dt.float32, tag=f"wsel{s}", bufs=1)
            nc.sync.dma_start(wtile[C:C + PC, :], eye3[:])
            wsel_sb.append(wtile)

        # ---- phase A: per-sample extended Gram accumulation
        # BW: weight-block width, padded to 128 so LDWEIGHTS gets FastWeightLoad
        BW = 128
        gps = []
        for s in range(SPC):
            gtile = gpool.tile([BW, AR], dt.float32, tag=f"G{s % 2}")
            gps.append(gtile)

        # per-sample distance prep, emitted right after sample s's Gram so it
        # hides under the next sample's phase-A work (psum scratch borrows the
        # phase-C bank, idle during phase A)
        qtce_box = {}

        def emit_b_sample(s):
            gsb = spool.tile([AR, AR], dt.float32, tag="gsb", name=f"gsb{s}")
            nc.vector.tensor_copy(gsb[:], gps[s][0:AR, :])
            if s == 0:
                qtce_box["t"] = gpool.tile([SPC, 195], dt.float32, tag="QT",
                                           name="qtce")
            qtce = qtce_box["t"]
            m1ps = cpool.tile([C, K * PC], dt.float32, tag="pp", name=f"m1ps{s}")
            nc.tensor.matmul(m1ps[:], gsb[0:C, 0:C], wtsb[:], start=True, stop=True)
            m1sb = spool.tile([C, K * PC], dt.float32, tag="m1sb", name=f"m1sb{s}")
            nc.vector.tensor_copy(m1sb[:], m1ps[:])
            ub = spool.tile([C, PC], dt.float32, tag="ub", name=f"ub{s}")
            nc.vector.tensor_tensor(ub[:], gsb[0:C, C:C + PC], gsb[0:C, C + PC:AR],
                                    op=Alu.subtract)
            eevps = cpool.tile([2 * PC, PC], dt.float32, tag="pp", name=f"eevps{s}")
            nc.tensor.matmul(eevps[:], gsb[C:AR, C:AR], vp64[C:AR, :],
                             start=True, stop=True)
            eevsb = spool.tile([2 * PC, PC], dt.float32, tag="eevsb", name=f"eevsb{s}")
            nc.vector.tensor_copy(eevsb[:], eevps[:])
            eevv = spool.tile([2 * PC, PC], dt.float32, tag="eevv", name=f"eevv{s}")
            nc.vector.tensor_tensor(eevv[:], eevsb[:], vp0[:], op=Alu.mult)
            tm3 = spool.tile([C, 195], dt.float32, tag="tm3", name=f"tm3{s}")
            ub_b = ub[:].unsqueeze(1).broadcast_to([C, K, PC])
            nc.vector.scalar_tensor_tensor(
                tm3[:, 0:K * PC], ub_b, 2.0, m1sb[:], op0=Alu.mult, op1=Alu.add)
            nc.vector.tensor_tensor(tm3[:, 0:K * PC], tm3[:, 0:K * PC], wtsb[:],
                                    op=Alu.mult)
            nc.vector.memset(tm3[:, K * PC:195], 0.0)
            nc.vector.tensor_copy(tm3[0:2 * PC, K * PC:195], eevv[:])
            nc.tensor.matmul(qtce[:], sel4sb[:, SPC * s:SPC * (s + 1)], tm3[:],
                             start=(s == 0), stop=(s == SPC - 1))

        copy_cnt = [0]
        with nc.named_scope("phaseA"):
            for s in range(SPC):
                for sc in range(NSC):
                    a_t = apool.tile([128, SC], dt.float16, tag="A")
                    # Alternate gpsimd/scalar queues (both idle during phase
                    # A, and off the Sync FIFO that phase-C loads can block).
                    # a_hi is host-padded to 128 rows: full-partition DMAs
                    # engage all 16 SDMA ports (~4x the 70-partition rate).
                    if sc % 2 == 0:
                        nc.gpsimd.dma_start(a_t[:], a_hi[s][:, sc * SC:(sc + 1) * SC])
                    else:
                        nc.scalar.dma_start(a_t[:], a_hi[s][:, sc * SC:(sc + 1) * SC])
                    for q in range(CPS // 8):
                        # 8 transposed chunks fill the 2KB psum bank exactly;
                        # halves the copy count vs 4-chunk batches
                        p_t = ppool.tile([128, 8 * BW], gdt, tag="pT")
                        for j in range(8):
                            ch = q * 8 + j
                            nc.tensor.transpose(
                                p_t[:, j * BW:j * BW + AR],
                                a_t[:, ch * 128:(ch + 1) * 128], id70[:])
                        at_t = atpool.tile([128, 8 * BW], gdt, tag="AT")
                        # alternate the PSUM->SBUF copies between DVE and ACT
                        # so neither engine sits on the critical chain alone
                        copy_cnt[0] += 1
                        if copy_cnt[0] % 3 != 0:
                            nc.vector.tensor_copy(at_t[:], p_t[:])
                        else:
                            nc.scalar.copy(at_t[:], p_t[:])
                        first = (sc == 0 and q == 0)
                        last = (sc == NSC - 1 and q == CPS // 8 - 1)
                        for j in range(8):
                            nc.tensor.matmul(
                                gps[s][0:BW, :],
                                at_t[:, j * BW:(j + 1) * BW],
                                at_t[:, j * BW:j * BW + AR],
                                start=(first and j == 0),
                                stop=(last and j == 7))
                emit_b_sample(s)

        # ---- phase B: distances, argmin, Wsel
        with nc.named_scope("phaseB"):
            qtce = qtce_box["t"]
            qkce = spool.tile([SPC, 65], dt.float32, tag="qkce")
            nc.vector.tensor_reduce(qkce[:], qtce[:].rearrange("a (k p) -> a k p", p=PC),
                                    axis=mybir.AxisListType.X, op=Alu.add)
            dneg = spool.tile([SPC, K], dt.float32, tag="dneg")
            nc.vector.tensor_scalar(dneg[:], qkce[:, 0:K], qkce[:, K:K + 1],
                                    -1.0 / NMEAN, op0=Alu.add, op1=Alu.mult)
            dmsb = spool.tile([SPC, K], dt.float32, tag="dmsb")
            nc.vector.tensor_scalar(dmsb[:], dneg[:], -1.0, None, op0=Alu.mult)
            nc.gpsimd.dma_start(dm_o[:], dmsb[:])

            mx8 = spool.tile([SPC, 8], dt.float32, tag="mx8")
            ix8 = spool.tile([SPC, 8], dt.uint32, tag="ix8")
            nc.vector.max(mx8[:], dneg[:])
            nc.vector.max_index(ix8[:], mx8[:], dneg[:])
            mn = spool.tile([SPC, 1], dt.float32, tag="mn")
            nc.vector.tensor_scalar(mn[:], mx8[:, 0:1], -1.0, None, op0=Alu.mult)
            nc.gpsimd.dma_start(loss_o[:], mn[:])
            nc.gpsimd.dma_start(dist_o[:], mn[:])
            idx32 = spool.tile([SPC, 1], dt.int32, tag="idx32")
            nc.vector.tensor_copy(idx32[:], ix8[:, 0:1])
            nc.gpsimd.dma_start(idx_o[:], idx32[:])

            oh = spool.tile([SPC, K], dt.float32, tag="oh")
            nc.vector.tensor_scalar(oh[:], dneg[:], mx8[:, 0:1], None, op0=Alu.is_equal)
            ohtps = ppool.tile([K, SPC], dt.float32, tag="pT")
            nc.tensor.transpose(ohtps[:], oh[:], id4[:])
            ohts = spool.tile([K, SPC], dt.float32, tag="ohts")
            nc.vector.tensor_copy(ohts[:], ohtps[:])

            # WselT per sample: 12 tiny matmuls into one bank
            wps = ppool.tile([C, PC * SPC], dt.float32, tag="pT")
            for s in range(SPC):
                for p in range(PC):
                    nc.tensor.matmul(wps[:, s * PC + p:s * PC + p + 1],
                                     wkpsb[:, p * C:(p + 1) * C],
                                     ohts[:, s:s + 1], start=True, stop=True)
            for s in range(SPC):
                nc.vector.tensor_copy(wsel_sb[s][0:C, :], wps[:, s * PC:(s + 1) * PC])

        # ---- phase C: predicts, fp32, col-tiled across the 4 samples
        with nc.named_scope("phaseC"):
            for sc in range(NSC):
                a2 = []
                for s in range(SPC):
                    a2t = a2pool.tile([C + PC, SC], dt.float32, tag=f"A2_{s}")
                    nc.sync.dma_start(a2t[0:C, :], feat[s][:, sc * SC:(sc + 1) * SC])
                    nc.sync.dma_start(a2t[C:C + PC, :], plq[s][:, sc * SC:(sc + 1) * SC])
                    a2.append(a2t)
                psb = spool.tile([99, SC], dt.float32, tag="psb")
                for w in range(SC // 512):
                    pp = cpool.tile([99, 512], dt.float32, tag="pp")
                    for s in range(SPC):
                        nc.tensor.matmul(
                            pp[32 * s:32 * s + PC, :], wsel_sb[s][:],
                            a2[s][:, w * 512:(w + 1) * 512],
                            start=True, stop=True, tile_position=(0, 32 * s),
                            skip_group_check=True)
                    nc.vector.tensor_copy(psb[:, w * 512:(w + 1) * 512], pp[:])
                off = sc * SC
                for s in range(SPC):
                    nc.scalar.dma_start(pred_o[s][:, off:off + SC],
                                        psb[32 * s:32 * s + PC, :])

    nc.compile()
    return nc


def _get_nc():
    if "nc" not in _CACHE:
        _CACHE["nc"] = _build_nc()
    return _CACHE["nc"]


def kernel(feat_last, target, predict_last, weight):
    feat_last = np.ascontiguousarray(np.asarray(feat_last, dtype=np.float32))
    target = np.ascontiguousarray(np.asarray(target, dtype=np.float32))
    predict_last = np.ascontiguousarray(np.asarray(predict_last, dtype=np.float32))
    weight = np.ascontiguousarray(np.asarray(weight, dtype=np.float32))

    featr = feat_last.reshape(B, C, HW)
    plr = predict_last.reshape(B, PC, HW)
    a_hi = np.zeros((B, 128, HW), dtype=np.float16)
    a_hi[:, 0:C] = featr
    a_hi[:, C:C + PC] = plr
    a_hi[:, C + PC:AR] = target.reshape(B, PC, HW)

    wt_np = np.ascontiguousarray(weight.T)                       # (64, 192)
    wkp_np = np.ascontiguousarray(
        weight.reshape(K, PC, C).transpose(1, 0, 2))             # (3, 64, 64)
    sel4_np = np.zeros((C, 4 * SPC), dtype=np.float32)
    for s in range(SPC):
        sel4_np[:, SPC * s + s] = 1.0
    vp_np = np.concatenate([np.eye(PC), -np.eye(PC)], axis=0).astype(np.float32)
    eye3_np = np.eye(PC, dtype=np.float32)

    in_maps = []
    for c in range(NCORES):
        sl = slice(c * SPC, (c + 1) * SPC)
        in_maps.append({
            "a_hi": np.ascontiguousarray(a_hi[sl]),
            "feat": np.ascontiguousarray(featr[sl]),
            "plq": np.ascontiguousarray(plr[sl]),
            "wt": wt_np, "wkp": wkp_np, "sel4": sel4_np,
            "vp": vp_np, "eye3": eye3_np,
        })

    nc = _get_nc()
    trace = os.environ.get("KERNEL_TRACE", "0") == "1"
    res = None
    last_err = None
    for attempt in range(3):
        try:
            res = run_bass_kernel_spmd(nc, in_maps, list(range(NCORES)),
                                       trace=trace,
                                       trace_cores=[0] if trace else None)
            break
        except Exception as e:  # transient NRT_EXEC_UNIT_UNRECOVERABLE right after
            last_err = e        # another process released the device; retry
            import time as _time
            _time.sleep(3.0 * (attempt + 1))
    if res is None:
        raise last_err
    if trace and res.exec_time_ns is not None:
        print(f"HW exec time: {res.exec_time_ns} ns")
        _CACHE["last_result"] = res

    predicts = np.concatenate([r["pred"] for r in res.results], axis=0)
    dm = np.concatenate([r["dm"] for r in res.results], axis=0)
    idx = np.concatenate([r["idx"][:, 0] for r in res.results], axis=0).astype(np.int32)
    loss = np.concatenate([r["loss"][:, 0] for r in res.results], axis=0)
    dists = np.concatenate([r["dists"][:, 0] for r in res.results], axis=0)
    return (predicts.reshape(B, PC, H, W_IMG), dm, idx, loss, dists)
